# revision 58
# baseline (speedup 1.0000x reference)
"""B3-spline undecimated wavelet transform (3 levels, reflect BC) on 8 trn2 cores.

Strategy
--------
Pure data parallel: 16 images -> 2 images per core.

Per level the separable 5-tap conv y = K_d @ Y @ K_d^T is computed as two
TensorEngine passes that each convolve along the *partition* axis and
transpose "for free":

    pass1:  AT = (K @ Y)^T      matmul(lhsT=Y_block, rhs=K^T_block)
    pass2:  Ynew = (K @ AT)^T   matmul(lhsT=AT_block, rhs=K^T_block)

K_d is banded (halfwidth 2d <= 8), so for each 128-row contraction block cb
only a narrow output window [cb*128-hw, cb*128+128+hw) is nonzero; each
window is issued as 1-2 matmuls (split at the 512-col PSUM bank boundaries)
accumulating into PSUM via the per-element has_written bits.

The data plane is fp16 end to end: x lives in HBM as fp16 (the host casts
once, outside the timed kernel), every SBUF tile is fp16, and the 4 output
planes are stored fp16 and upcast to f32 on the host. The conv weights are
dyadic rationals and exact in fp16; accumulation is fp32 in PSUM;
end-to-end error is ~5e-4 vs the f32 reference.

Engine budget (all HW-measured, the cost model is badly off on trn2):
the PE instruction stream is only ~10us/rep, so the kernel is paced by
the PSUM evacuations (f32 psum -> f16 SBUF). Only ACT/DVE can touch PSUM
(GPSIMD cannot, and is 2.3x slower than modeled anyway), so PSUM tiles
are PAIRED [128, 2048] (4 banks, ring of 2) and each evac is one big
instruction amortizing the ~0.35us per-instruction sync/access overhead;
the 48 evac pairs are split ~38/10 between ACT and DVE. The wavelet
subtraction w_i = m_{i-1} - m_i runs as half-plane [128, 4096] all-f16
SBUF ops on DVE (m tiles are half-plane sized so 12 quad-subs replace
24 pair-subs). The two images per core are interleaved level-by-level
so one image's pass-1 matmuls hide the other's evac barrier. HBM DMA is
spread over three DGE queues: SP carries x loads + half the w stores,
the gpsimd SWDGE ring the other w half + c3; the ACT engine issues no
DMA so its sequencer stays on evacuations.

Measured: ~39us/repeat (median pair-slope; quiet-machine slopes ~27-35)
vs the 117us f32 baseline (rel err 3.3e-4).
"""

import sys

if "/opt/trn_rl_repo" not in sys.path:
    sys.path.insert(0, "/opt/trn_rl_repo")

import numpy as np

import concourse.bass as bass
import concourse.mybir as mybir
import concourse.tile as tile
from concourse import bacc
from concourse.bass_utils import run_bass_kernel_spmd

P = 128
L = 1024
NB = L // P            # 8 blocks per axis
NQ = NB // 2           # 4 block-pairs per axis
BPC = 2                # images per core
NCORES = 8
LEVELS = (1, 2, 4)     # dilation per level
F32 = mybir.dt.float32
F16 = mybir.dt.float16
W5 = (1.0 / 16, 1.0 / 4, 3.0 / 8, 1.0 / 4, 1.0 / 16)

# --- engine assignment (tunable) -------------------------------------------
# HW-measured costs: ACT psum single-evac ~0.33us, DVE ~0.65us, DVE f16
# pair-sub ~0.6us, Pool f16 pair-sub 4.2us (gpsimd stays off the critical
# path; its SWDGE ring carries DMA instead). The PE instruction stream
# itself is only ~10us/rep, so the psum-evac pipeline and the DMA queues
# set the pace.
# evac engine per (level, pass): 4 pair-evacs per pass, s=ACT, v=DVE.
# Pairs amortize the ~0.35us per-instruction sync/access overhead that
# doubles the cost of single-tile evacs in-kernel.
EVAC_ENG = {
    (0, 0): ("s", "s", "s", "s"),
    (0, 1): ("s", "v", "s", "s"),
    (1, 0): ("s", "s", "v", "s"),
    (1, 1): ("s", "v", "s", "s"),
    (2, 0): ("s", "s", "v", "s"),
    (2, 1): ("s", "v", "s", "s"),
}
# wavelet-subtraction engine per image half (all-f16 sbuf operands).
SUB_ENG = ("v", "v")
# HWDGE/SWDGE queue per w-store (level, half), for c3 halves, and x loads.
STORE_Q = {
    (0, 0): "sp", (0, 1): "g",
    (1, 0): "sp", (1, 1): "g",
    (2, 0): "sp", (2, 1): "g",
}
C3_Q = {0: "g", 1: "g"}
LOAD_Q = {0: "sp", 1: "sp"}
# SBUF pool sizes (tiles): x halves 1MiB, a pairs 0.5MiB, m halves 1MiB,
# w halves 1MiB, c3 halves 1MiB. Total must stay under ~26MiB. A/B-tested:
# w=5/c3=3 staging slack beats extra m or x prefetch buffers.
POOL_BUFS = {"x": 4, "a": 9, "m": 8, "w": 5, "c3": 3}


def _conv_matrix(d: int) -> np.ndarray:
    """K such that (K @ x) == dilated reflect-padded 5-tap conv along axis 0."""
    eye = np.eye(L, dtype=np.float64)
    xp = np.pad(eye, ((2 * d, 2 * d), (0, 0)), mode="reflect")
    K = np.zeros((L, L), dtype=np.float64)
    for k in range(5):
        K += W5[k] * xp[k * d : k * d + L]
    return K.astype(np.float32)


def _const_arrays() -> dict[str, np.ndarray]:
    """fp16 K^T blocks per level: interior Toeplitz block + the two edge blocks."""
    consts = {}
    for li, d in enumerate(LEVELS):
        hw = 2 * d
        KT = _conv_matrix(d).T  # KT[i, n] = K[n, i]
        kint = KT[P : 2 * P, P - hw : 2 * P + hw]
        k0 = KT[0:P, 0 : P + hw]
        k7 = KT[7 * P : 8 * P, 7 * P - hw : 8 * P]
        for nm, a in ((f"kint{li}", kint), (f"k0{li}", k0), (f"k7{li}", k7)):
            a16 = np.ascontiguousarray(a, dtype=np.float16)
            assert np.array_equal(a16.astype(np.float32), a.astype(np.float32))
            consts[nm] = a16
    return consts


def _windows(li: int, cb: int):
    """Nonzero output-column segments for contraction block cb, split at the
    PSUM bank boundary. Returns [(c0, c1, const_name, rhs_col_offset)]."""
    hw = 2 * LEVELS[li]
    if cb == 0:
        c0, c1, nm, base = 0, P + hw, f"k0{li}", 0
    elif cb == NB - 1:
        c0, c1, nm, base = 7 * P - hw, L, f"k7{li}", 7 * P - hw
    else:
        c0, c1, nm, base = cb * P - hw, cb * P + P + hw, f"kint{li}", cb * P - hw
    segs = [(c0, 512), (512, c1)] if c0 < 512 < c1 else [(c0, c1)]
    return [(a, b, nm, a - base) for a, b in segs]


def _mm_list(li: int):
    """Ordered matmul segments for one 1024-col output block with per-bank
    start/stop flags (banks 0/1 local to the block)."""
    segs = []
    for cb in range(NB):
        for a, b, nm, off in _windows(li, cb):
            segs.append([cb, a, b, nm, off, False, False])
    first, last = {}, {}
    for i, s in enumerate(segs):
        bank = s[1] // 512
        first.setdefault(bank, i)
        last[bank] = i
    for i in first.values():
        segs[i][5] = True  # start: clears the bank's has_written bits
    for i in last.values():
        segs[i][6] = True  # stop: closes the accumulation group
    return [tuple(s) for s in segs]


def _blk(pairs, cb, c0, c1):
    """[P, c1-c0] AP of block cb's columns [c0, c1) from pair APs [P, 2L]."""
    base = (cb % 2) * L
    return pairs[cb // 2][:, base + c0 : base + c1]


def _conv_pass(nc, ksb, src_pairs, segs, pspool, consume):
    """One transposing conv pass: src pair APs -> 4 PSUM pair tiles [P, 2L]."""
    for q in range(NQ):
        ps = pspool.tile([P, 2 * L], F32, tag="ps", name="ps")
        for half in range(2):
            mb = 2 * q + half
            for cb, a, b, nm, off, st, sp in segs:
                nc.tensor.matmul(
                    ps[:, half * L + a : half * L + b],
                    _blk(src_pairs, cb, mb * P, (mb + 1) * P),
                    ksb[nm][:, off : off + (b - a)],
                    start=st,
                    stop=sp,
                )
        consume(q, ps)


def _pair_ap(halves, q):
    """[P, 2L] AP for pair q from two [P, 4, L] half-plane tiles."""
    h, r = divmod(q, 2)
    return halves[h][:, 2 * r : 2 * r + 2, :].rearrange("p b w -> p (b w)")


def _build_nc(repeat: int = 1, probe: int = 0):
    """probe=0: full kernel. probe=1: no subs / w stores (timing probe).
    probe=2: like 1 but all evacs on ACT. probe=3: matmuls + 1-col psum
    drains only, all passes read the x tiles (results garbage; isolates the
    PE instruction stream)."""
    consts = _const_arrays()
    nc = bacc.Bacc(
        "TRN2",
        target_bir_lowering=False,
        debug=False,
        num_devices=NCORES,
    )
    x_in = nc.dram_tensor("x", [BPC, L, L], F16, kind="ExternalInput")
    out = nc.dram_tensor("out", [BPC, 4, L, L], F16, kind="ExternalOutput")
    knames = list(consts)
    kwidths = [consts[nm].shape[1] for nm in knames]
    koffs = dict(zip(knames, np.cumsum([0] + kwidths[:-1]).tolist()))
    ktotal = int(sum(kwidths))
    kall = nc.dram_tensor("kall", [P, ktotal], F16, kind="ExternalInput")

    with tile.TileContext(nc) as tc:
        with (
            tc.tile_pool(name="consts", bufs=1) as cpool,
            tc.tile_pool(name="xin", bufs=POOL_BUFS["x"]) as xpool,
            tc.tile_pool(name="apool", bufs=POOL_BUFS["a"]) as apool,
            tc.tile_pool(name="mpool", bufs=POOL_BUFS["m"]) as mpool,
            tc.tile_pool(name="wout", bufs=POOL_BUFS["w"]) as wpool,
            tc.tile_pool(name="c3out", bufs=POOL_BUFS["c3"]) as c3pool,
            tc.tile_pool(name="ps", bufs=2, space="PSUM") as pspool,
        ):
            kall_sb = cpool.tile([P, ktotal], F16, name="kall_sb")
            ksb = {
                nm: kall_sb[:, koffs[nm] : koffs[nm] + consts[nm].shape[1]]
                for nm in knames
            }

            def evac(e, dst_ap, ps):
                if probe == 3:
                    nc.scalar.copy(dst_ap[:, 0:1], ps[:, 0:1])
                    return
                if probe == 2:
                    e = "s"
                if e == "s":
                    nc.scalar.copy(dst_ap, ps[:, :])
                else:
                    nc.vector.tensor_copy(dst_ap, ps[:, :])

            sub_eng = {"v": nc.vector, "g": nc.gpsimd}
            dma_q = {"sp": nc.sync, "act": nc.scalar, "g": nc.gpsimd}

            kall_loaded = False
            half = P * NB // 2
            for _rep in range(repeat):
                # The two images are interleaved LEVEL BY LEVEL: image B's
                # pass-1 matmuls run while image A's pass-2 waits on A's
                # pass-1 evacuations, hiding the per-pass evac barrier that
                # the transposing contraction imposes (every pass-2 output
                # tile reads all 8 pass-1 evac blocks).
                x_halves, m_prev = {}, {}
                for img in range(BPC):
                    x_halves[img] = []
                    for h in range(2):
                        xt = xpool.tile([P, NB // 2, L], F16, tag="x", name="x_sb")
                        dma_q[LOAD_Q[h]].dma_start(
                            xt[:],
                            x_in[img, h * half : (h + 1) * half].rearrange(
                                "(b p) w -> p b w", p=P
                            ),
                        )
                        x_halves[img].append(xt)
                        if not kall_loaded:
                            nc.scalar.dma_start(kall_sb[:], kall[:, :])
                            kall_loaded = True
                    m_prev[img] = (
                        [_pair_ap(x_halves[img], q) for q in range(NQ)],
                        [
                            xt[:].rearrange("p b w -> p (b w)")
                            for xt in x_halves[img]
                        ],
                    )

                for li in range(len(LEVELS)):
                    segs = _mm_list(li)
                    last = li == len(LEVELS) - 1
                    at_pairs = {}

                    for img in range(BPC):
                        # pass 1: AT = (K @ Y)^T -> f16 pair tiles
                        at_pairs[img] = [
                            apool.tile([P, 2 * L], F16, tag="a", name="at")[:, :]
                            for _ in range(NQ)
                        ]

                        def evac_at(q, ps, at=at_pairs[img], li=li):
                            evac(EVAC_ENG[(li, 0)][q], at[q], ps)

                        _conv_pass(
                            nc,
                            ksb,
                            m_prev[img][0]
                            if probe != 3
                            else [_pair_ap(x_halves[img], q) for q in range(NQ)],
                            segs,
                            pspool,
                            evac_at,
                        )

                    for img in range(BPC):
                        # pass 2: Ynew -> f16 m pair tiles (level 3: straight
                        # into the c3 half-plane staging tiles). Wavelet sub
                        # w = m_prev - m on all-f16 SBUF pairs, then
                        # half-plane stores stream out.
                        w_halves = (
                            [
                                wpool.tile([P, NB // 2, L], F16, tag="w", name="w_sb")
                                for _ in range(2)
                            ]
                            if probe == 0
                            else None
                        )
                        if last:
                            c3_halves = [
                                c3pool.tile(
                                    [P, NB // 2, L], F16, tag="c3", name="c3_sb"
                                )
                                for _ in range(2)
                            ]
                            m_cur_p = [_pair_ap(c3_halves, q) for q in range(NQ)]
                            m_cur_h = [
                                c3_halves[h][:].rearrange("p b w -> p (b w)")
                                for h in range(2)
                            ]
                        else:
                            m_tiles = [
                                mpool.tile([P, 4 * L], F16, tag="m", name="m_sb")
                                for _ in range(2)
                            ]
                            m_cur_p = [
                                m_tiles[q // 2][
                                    :, (q % 2) * 2 * L : (q % 2 + 1) * 2 * L
                                ]
                                for q in range(NQ)
                            ]
                            m_cur_h = [t[:, :] for t in m_tiles]

                        def evac_m(
                            q,
                            ps,
                            m_cur_p=m_cur_p,
                            m_cur_h=m_cur_h,
                            m_prev=m_prev[img],
                            w=w_halves,
                            li=li,
                        ):
                            evac(EVAC_ENG[(li, 1)][q], m_cur_p[q], ps)
                            if probe == 0 and q % 2 == 1:
                                h = q // 2
                                sub_eng[SUB_ENG[h]].tensor_sub(
                                    w[h][:].rearrange("p b w -> p (b w)"),
                                    m_prev[1][h],
                                    m_cur_h[h],
                                )

                        _conv_pass(
                            nc,
                            ksb,
                            at_pairs[img]
                            if probe != 3
                            else [_pair_ap(x_halves[img], q) for q in range(NQ)],
                            segs,
                            pspool,
                            evac_m,
                        )
                        m_cur = (m_cur_p, m_cur_h)

                        for h in range(2):
                            if probe == 0:
                                dma_q[STORE_Q[(li, h)]].dma_start(
                                    out[
                                        img, li, h * half : (h + 1) * half
                                    ].rearrange("(b p) w -> p b w", p=P),
                                    w_halves[h][:],
                                )
                            if last and probe == 0:
                                dma_q[C3_Q[h]].dma_start(
                                    out[
                                        img, 3, h * half : (h + 1) * half
                                    ].rearrange("(b p) w -> p b w", p=P),
                                    c3_halves[h][:],
                                )
                        if not last:
                            m_prev[img] = m_cur
    nc.compile()
    return nc


def _kall_array() -> np.ndarray:
    consts = _const_arrays()
    return np.ascontiguousarray(
        np.concatenate([consts[nm] for nm in consts], axis=1), dtype=np.float16
    )


_NC_CACHE = None


def _get_nc():
    global _NC_CACHE
    if _NC_CACHE is None:
        _NC_CACHE = _build_nc()
    return _NC_CACHE


def _in_maps(x: np.ndarray) -> list[dict[str, np.ndarray]]:
    assert x.shape == (BPC * NCORES, L, L), x.shape
    x16 = np.ascontiguousarray(x, dtype=np.float16)
    kall = _kall_array()
    return [
        {
            "x": np.ascontiguousarray(x16[c * BPC : (c + 1) * BPC]),
            "kall": kall,
        }
        for c in range(NCORES)
    ]


def _run(x: np.ndarray, **spmd_kwargs):
    nc = _get_nc()
    in_maps = _in_maps(x)
    res = run_bass_kernel_spmd(nc, in_maps, core_ids=list(range(NCORES)), **spmd_kwargs)
    full = np.concatenate([res.results[c]["out"] for c in range(NCORES)], axis=0)
    return full, res


def kernel(x: np.ndarray) -> np.ndarray:
    full, _ = _run(x)
    return full.astype(np.float32)
